# revision 6
# baseline (speedup 1.0000x reference)
"""Multi-head attention (B=2, S=2048, D=1024, H=16) on 8 TRN2 NeuronCores.

Sharding: batch*heads across cores — core c handles batch c//4, heads
4*(c%4) .. 4*(c%4)+4. Each core computes its 4 heads' projections, full
softmax attention (written to HBM as the attn output shard), and a partial
output projection; the host sums the 4 partial outputs per batch and adds bo.

Per-core device program (identical SPMD program, per-core data):
  Stage A: PE-transpose inputs tile-by-tile, project q/k/v as [head_dim, seq]
           (f32r matmuls, head-pair packed M=128), second transpose for v.
  Per head: Phase 1: scores[q,k] -> ScalarE Exp (accum_out row sums) ->
            reciprocal -> in-place normalize -> DMA attn shard.
            Phase 2: scores_T[k,q] -> Exp -> PV matmul (v stationary) ->
            av[64, S] PSUM; normalize via transpose/scale/transpose-back.
  Stage C: output projection with WoT, partial out -> HBM.
"""

import numpy as np

B = 2
S = 2048
D = 1024
H = 16
DH = 64
HPC = 4  # heads per core
N_CORES = 8
SCALE = 1.0 / 8.0  # 1/sqrt(DH), folded into Wq/bq on host

_CACHE = {}


def _build():
    import concourse.bass as bass  # noqa: F401
    import concourse.mybir as mybir
    import concourse.tile as tile
    from concourse import bacc
    from concourse.masks import make_identity

    F32 = mybir.dt.float32
    F32R = mybir.dt.float32r
    EXP = mybir.ActivationFunctionType.Exp

    nc = bacc.Bacc("TRN2", target_bir_lowering=False, debug=False)

    xq_d = nc.dram_tensor("xq", [S, D], F32R, kind="ExternalInput").ap()
    xk_d = nc.dram_tensor("xk", [S, D], F32R, kind="ExternalInput").ap()
    xv_d = nc.dram_tensor("xv", [S, D], F32R, kind="ExternalInput").ap()
    wqT_d = nc.dram_tensor("wqT", [D, HPC * DH], F32R, kind="ExternalInput").ap()
    wkT_d = nc.dram_tensor("wkT", [D, HPC * DH], F32R, kind="ExternalInput").ap()
    wvT_d = nc.dram_tensor("wvT", [D, HPC * DH], F32R, kind="ExternalInput").ap()
    woT_d = nc.dram_tensor("woT", [HPC * DH, D], F32R, kind="ExternalInput").ap()
    bq_d = nc.dram_tensor("bq", [HPC * DH], F32, kind="ExternalInput").ap()
    bk_d = nc.dram_tensor("bk", [HPC * DH], F32, kind="ExternalInput").ap()
    bv_d = nc.dram_tensor("bv", [HPC * DH], F32, kind="ExternalInput").ap()

    attn_d = nc.dram_tensor("attn", [HPC, S, S], F32, kind="ExternalOutput").ap()
    pout_d = nc.dram_tensor("pout", [S, D], F32, kind="ExternalOutput").ap()

    NQT = S // 128  # 16 query tiles of 128 rows

    with tile.TileContext(nc) as tc:
        import contextlib

        with contextlib.ExitStack() as ctx:
            const = ctx.enter_context(tc.tile_pool(name="const", bufs=1))
            persist = ctx.enter_context(tc.tile_pool(name="persist", bufs=1))

            ident_f = const.tile([128, 128], F32, name="ident_f")
            make_identity(nc, ident_f[:])
            ident_r = const.tile([128, 128], F32R, name="ident_r")
            nc.vector.tensor_copy(ident_r[:], ident_f[:])

            # persistent tensors
            qT = [persist.tile([128, S], F32R, name=f"qT{hp}") for hp in range(2)]
            kT = [persist.tile([128, S], F32R, name=f"kT{hp}") for hp in range(2)]
            v_sb = persist.tile([128, (S // 128) * HPC * DH], F32R, name="v_sb")
            avT = [persist.tile([64, S], F32R, name=f"avT{h}") for h in range(HPC)]
            rsum = persist.tile([128, HPC * NQT], F32, name="rsum")
            woT_sb = persist.tile([64, HPC, D], F32R, name="woT_sb")
            b_sb = {}
            for nm, bd in (("q", bq_d), ("k", bk_d), ("v", bv_d)):
                t = persist.tile([128, 2], F32, name=f"b{nm}_sb")
                nc.sync.dma_start(t[:], bd.rearrange("(hp p) -> p hp", p=128))
                b_sb[nm] = t
            nc.sync.dma_start(
                woT_sb[:], woT_d.rearrange("(hh p) j -> p hh j", p=64)
            )

            # ---------------- Stage A: transpose + projections ----------------
            with contextlib.ExitStack() as actx:
                nat_pool = actx.enter_context(tc.tile_pool(name="nat", bufs=2))
                xt_pool = actx.enter_context(tc.tile_pool(name="xt", bufs=3))
                wT_pool = actx.enter_context(tc.tile_pool(name="wT", bufs=1))
                vtmp_pool = actx.enter_context(tc.tile_pool(name="vtmp", bufs=1))
                tp_ps = actx.enter_context(
                    tc.tile_pool(name="tp_ps", bufs=2, space="PSUM")
                )
                pr_ps = actx.enter_context(
                    tc.tile_pool(name="pr_ps", bufs=2, space="PSUM")
                )

                vT_tmp = [
                    vtmp_pool.tile([128, S], F32R, name=f"vT{hp}", tag=f"vT{hp}")
                    for hp in range(2)
                ]

                for x_d, wT_d2, bias_nm, dests in (
                    (xq_d, wqT_d, "q", qT),
                    (xk_d, wkT_d, "k", kT),
                    (xv_d, wvT_d, "v", vT_tmp),
                ):
                    wT_sb = wT_pool.tile([128, 8, HPC * DH], F32R, tag="wT", name="wT_sb")
                    nc.sync.dma_start(
                        wT_sb[:], wT_d2.rearrange("(dt p) c -> p dt c", p=128)
                    )
                    for sc in range(4):  # 512-row s-chunks
                        nat = nat_pool.tile([128, 4, D], F32R, tag="nat", name="nat")
                        nc.sync.dma_start(
                            nat[:],
                            x_d[sc * 512 : (sc + 1) * 512, :].rearrange(
                                "(so p) d -> p so d", p=128
                            ),
                        )
                        pps = [
                            pr_ps.tile([128, 512], F32, tag="pr", name="pps")
                            for _ in range(2)
                        ]
                        for dt in range(8):
                            tp = tp_ps.tile([128, 512], F32R, tag="tp", name="tp")
                            for so in range(4):
                                nc.tensor.transpose(
                                    tp[:, so * 128 : (so + 1) * 128],
                                    nat[:, so, dt * 128 : (dt + 1) * 128],
                                    ident_r[:],
                                )
                            xt = xt_pool.tile([128, 512], F32R, tag="xt", name="xt")
                            nc.vector.tensor_copy(xt[:], tp[:])
                            for hp in range(2):
                                nc.tensor.matmul(
                                    pps[hp][:],
                                    wT_sb[:, dt, hp * 128 : (hp + 1) * 128],
                                    xt[:],
                                    start=(dt == 0),
                                    stop=(dt == 7),
                                )
                        for hp in range(2):
                            nc.vector.tensor_scalar_add(
                                dests[hp][:, sc * 512 : (sc + 1) * 512],
                                pps[hp][:],
                                b_sb[bias_nm][:, hp : hp + 1],
                            )

                # v: second transpose into natural [seq, head_dim] layout
                for hp in range(2):
                    for st in range(S // 128):
                        tpv = tp_ps.tile([128, 512], F32R, tag="tp", name="tpv")
                        nc.tensor.transpose(
                            tpv[:, :128],
                            vT_tmp[hp][:, st * 128 : (st + 1) * 128],
                            ident_r[:],
                        )
                        nc.vector.tensor_copy(
                            v_sb[:, st * 256 + hp * 128 : st * 256 + (hp + 1) * 128],
                            tpv[:, :128],
                        )

            # ---------------- Stage B: attention per head ----------------
            with contextlib.ExitStack() as bctx:
                p_pool = bctx.enter_context(tc.tile_pool(name="p", bufs=3))
                pT_pool = bctx.enter_context(tc.tile_pool(name="pT", bufs=3))
                sm_pool = bctx.enter_context(tc.tile_pool(name="sm", bufs=8))
                avsb_pool = bctx.enter_context(tc.tile_pool(name="avsb", bufs=2))
                avn_pool = bctx.enter_context(tc.tile_pool(name="avn", bufs=3))
                out_pool = bctx.enter_context(tc.tile_pool(name="outp", bufs=2))
                sc_ps = bctx.enter_context(
                    tc.tile_pool(name="sc_ps", bufs=2, space="PSUM")
                )
                av_ps_pool = bctx.enter_context(
                    tc.tile_pool(name="av_ps", bufs=1, space="PSUM")
                )

                for h in range(HPC):
                    hp, off = h // 2, (h % 2) * 64
                    qTh = qT[hp][off : off + 64, :]
                    kTh = kT[hp][off : off + 64, :]

                    # Phase 1: attn output rows
                    for qt in range(NQT):
                        p_sb = p_pool.tile([128, S], F32, tag="p", name="p_sb")
                        sums2 = sm_pool.tile([128, 2], F32, tag="sm", name="sums2")
                        for half in range(2):
                            sps = sc_ps.tile([128, 1024], F32, tag="sc", name="sps")
                            for nk in range(2):
                                nc.tensor.matmul(
                                    sps[:, nk * 512 : (nk + 1) * 512],
                                    qTh[:, qt * 128 : (qt + 1) * 128],
                                    kTh[:, half * 1024 + nk * 512 : half * 1024 + (nk + 1) * 512],
                                    start=True,
                                    stop=True,
                                )
                            nc.scalar.activation(
                                p_sb[:, half * 1024 : (half + 1) * 1024],
                                sps[:],
                                EXP,
                                accum_out=sums2[:, half : half + 1],
                            )
                        tot = sm_pool.tile([128, 1], F32, tag="tot", name="tot")
                        nc.vector.tensor_add(tot[:], sums2[:, 0:1], sums2[:, 1:2])
                        rs = rsum[:, h * NQT + qt : h * NQT + qt + 1]
                        nc.vector.reciprocal(rs, tot[:])
                        nc.vector.tensor_scalar_mul(p_sb[:], p_sb[:], rs)
                        nc.sync.dma_start(
                            attn_d[h, qt * 128 : (qt + 1) * 128, :], p_sb[:]
                        )

                    # Phase 2: PV product (unnormalized), av in [64, S] PSUM
                    av_ps = av_ps_pool.tile([64, S], F32, tag="av", name="av_ps")
                    for kt in range(S // 128):
                        pT = pT_pool.tile([128, S], F32R, tag="pT", name="pT")
                        for half in range(2):
                            sps = sc_ps.tile([128, 1024], F32, tag="sc", name="spsT")
                            for nq in range(2):
                                nc.tensor.matmul(
                                    sps[:, nq * 512 : (nq + 1) * 512],
                                    kTh[:, kt * 128 : (kt + 1) * 128],
                                    qTh[:, half * 1024 + nq * 512 : half * 1024 + (nq + 1) * 512],
                                    start=True,
                                    stop=True,
                                )
                            nc.scalar.activation(
                                pT[:, half * 1024 : (half + 1) * 1024], sps[:], EXP
                            )
                        vcol = kt * 256 + hp * 128 + (h % 2) * 64
                        for qc in range(4):
                            nc.tensor.matmul(
                                av_ps[:, qc * 512 : (qc + 1) * 512],
                                v_sb[:, vcol : vcol + 64],
                                pT[:, qc * 512 : (qc + 1) * 512],
                                start=(kt == 0),
                                stop=(kt == S // 128 - 1),
                                skip_group_check=True,
                            )

                    # normalize av rows by 1/rowsum via transpose round-trip
                    av_sb = avsb_pool.tile([64, S], F32R, tag="avsb", name="av_sb")
                    nc.vector.tensor_copy(av_sb[:], av_ps[:])
                    for qt in range(NQT):
                        t1 = sc_ps.tile([128, 1024], F32R, tag="sc", name="t1")
                        nc.tensor.transpose(
                            t1[:, :64],
                            av_sb[:, qt * 128 : (qt + 1) * 128],
                            ident_r[:64, :64],
                        )
                        avn = avn_pool.tile([128, 64], F32R, tag="avn", name="avn")
                        nc.vector.tensor_scalar_mul(
                            avn[:],
                            t1[:, :64],
                            rsum[:, h * NQT + qt : h * NQT + qt + 1],
                        )
                        t2 = sc_ps.tile([128, 1024], F32R, tag="sc", name="t2")
                        nc.tensor.transpose(
                            t2[:64, :128],
                            avn[:],
                            ident_r[:],
                        )
                        nc.vector.tensor_copy(
                            avT[h][:, qt * 128 : (qt + 1) * 128],
                            t2[:64, :128],
                        )

                # ---------------- Stage C: output projection ----------------
                for qt in range(NQT):
                    wps = sc_ps.tile([128, 1024], F32, tag="sc", name="wps")
                    for hh in range(HPC):
                        for nj in range(2):
                            nc.tensor.matmul(
                                wps[:, nj * 512 : (nj + 1) * 512],
                                avT[hh][:, qt * 128 : (qt + 1) * 128],
                                woT_sb[:, hh, nj * 512 : (nj + 1) * 512],
                                start=(hh == 0),
                                stop=(hh == HPC - 1),
                            )
                    out_sb = out_pool.tile([128, D], F32, tag="out", name="out_sb")
                    nc.vector.tensor_copy(out_sb[:], wps[:])
                    nc.sync.dma_start(pout_d[qt * 128 : (qt + 1) * 128, :], out_sb[:])

    nc.compile()
    return nc


def _get_nc():
    if "nc" not in _CACHE:
        _CACHE["nc"] = _build()
    return _CACHE["nc"]


def _in_maps(query, key, value, Wq, bq, Wk, bk, Wv, bv, Wo, bo):
    maps = []
    for c in range(N_CORES):
        b = c // 4
        h0 = (c % 4) * HPC
        cols = slice(h0 * DH, (h0 + HPC) * DH)
        maps.append(
            {
                "xq": np.ascontiguousarray(query[b]),
                "xk": np.ascontiguousarray(key[b]),
                "xv": np.ascontiguousarray(value[b]),
                "wqT": np.ascontiguousarray((Wq[cols] * SCALE).T),
                "wkT": np.ascontiguousarray(Wk[cols].T),
                "wvT": np.ascontiguousarray(Wv[cols].T),
                "woT": np.ascontiguousarray(Wo[:, cols].T),
                "bq": np.ascontiguousarray(bq[cols] * SCALE),
                "bk": np.ascontiguousarray(bk[cols]),
                "bv": np.ascontiguousarray(bv[cols]),
            }
        )
    return maps


def kernel(query, key, value, Wq, bq, Wk, bk, Wv, bv, Wo, bo, _return_results=False):
    from concourse.bass_utils import run_bass_kernel_spmd

    args = [np.asarray(a, dtype=np.float32) for a in
            (query, key, value, Wq, bq, Wk, bk, Wv, bv, Wo, bo)]
    query, key, value, Wq, bq, Wk, bk, Wv, bv, Wo, bo = args

    nc = _get_nc()
    maps = _in_maps(query, key, value, Wq, bq, Wk, bk, Wv, bv, Wo, bo)
    res = run_bass_kernel_spmd(nc, maps, core_ids=list(range(N_CORES)))

    attn = np.empty((B, H, S, S), dtype=np.float32)
    out = np.zeros((B, S, D), dtype=np.float32)
    for c in range(N_CORES):
        b = c // 4
        h0 = (c % 4) * HPC
        attn[b, h0 : h0 + HPC] = res.results[c]["attn"]
        out[b] += res.results[c]["pout"]
    out += bo
    if _return_results:
        return out, attn, res
    return out, attn


# revision 9
# speedup vs baseline: 29949.0771x; 29949.0771x over previous
"""Multi-head attention (B=2, S=2048, D=1024, H=16) on 8 TRN2 NeuronCores.

Sharding: batch*heads across cores — core c handles batch c//4, heads
4*(c%4) .. 4*(c%4)+4. Each core computes its 4 heads' projections, full
softmax attention (written to HBM as the attn output shard), and a partial
output projection; the host sums the 4 partial outputs per batch and adds bo.

Per-core device program (identical SPMD program, per-core data):
  Stage A: PE-transpose inputs tile-by-tile, project q/k/v as [head_dim, seq]
           (f32r matmuls, head-pair packed M=128), second transpose for v.
  Per head: Phase 1: scores[q,k] -> ScalarE Exp (accum_out row sums) ->
            reciprocal -> in-place normalize -> DMA attn shard.
            Phase 2: scores_T[k,q] -> Exp -> PV matmul (v stationary) ->
            av[64, S] PSUM; normalize via transpose/scale/transpose-back.
  Stage C: output projection with WoT, partial out -> HBM.
"""

import numpy as np

B = 2
S = 2048
D = 1024
H = 16
DH = 64
HPC = 4  # heads per core
N_CORES = 8
SCALE = 1.0 / 8.0  # 1/sqrt(DH), folded into Wq/bq on host

_CACHE = {}


def _build():
    import concourse.bass as bass  # noqa: F401
    import concourse.mybir as mybir
    import concourse.tile as tile
    from concourse import bacc
    from concourse.masks import make_identity

    F32 = mybir.dt.float32
    F32R = mybir.dt.float32r
    EXP = mybir.ActivationFunctionType.Exp

    nc = bacc.Bacc("TRN2", target_bir_lowering=False, debug=False)

    xq_d = nc.dram_tensor("xq", [S, D], F32R, kind="ExternalInput").ap()
    xk_d = nc.dram_tensor("xk", [S, D], F32R, kind="ExternalInput").ap()
    xv_d = nc.dram_tensor("xv", [S, D], F32R, kind="ExternalInput").ap()
    wqT_d = nc.dram_tensor("wqT", [D, HPC * DH], F32R, kind="ExternalInput").ap()
    wkT_d = nc.dram_tensor("wkT", [D, HPC * DH], F32R, kind="ExternalInput").ap()
    wvT_d = nc.dram_tensor("wvT", [D, HPC * DH], F32R, kind="ExternalInput").ap()
    woT_d = nc.dram_tensor("woT", [HPC * DH, D], F32R, kind="ExternalInput").ap()
    bq_d = nc.dram_tensor("bq", [HPC * DH], F32, kind="ExternalInput").ap()
    bk_d = nc.dram_tensor("bk", [HPC * DH], F32, kind="ExternalInput").ap()
    bv_d = nc.dram_tensor("bv", [HPC * DH], F32, kind="ExternalInput").ap()

    attn_d = nc.dram_tensor("attn", [HPC, S, S], F32, kind="ExternalOutput").ap()
    pout_d = nc.dram_tensor("pout", [S, D], F32, kind="ExternalOutput").ap()

    NQT = S // 128  # 16 query tiles of 128 rows

    with tile.TileContext(nc) as tc:
        import contextlib

        with contextlib.ExitStack() as ctx:
            const = ctx.enter_context(tc.tile_pool(name="const", bufs=1))
            persist = ctx.enter_context(tc.tile_pool(name="persist", bufs=1))

            ident_f = const.tile([128, 128], F32, name="ident_f")
            make_identity(nc, ident_f[:])
            ident_r = const.tile([128, 128], F32R, name="ident_r")
            nc.vector.tensor_copy(ident_r[:], ident_f[:])

            # persistent tensors
            qT = [persist.tile([128, S], F32R, name=f"qT{hp}") for hp in range(2)]
            kT = [persist.tile([128, S], F32R, name=f"kT{hp}") for hp in range(2)]
            v_sb = persist.tile([128, (S // 128) * HPC * DH], F32R, name="v_sb")
            avT = [persist.tile([64, S], F32R, name=f"avT{h}") for h in range(HPC)]
            rsum = persist.tile([128, HPC * NQT], F32, name="rsum")
            woT_sb = persist.tile([64, HPC, D], F32R, name="woT_sb")
            b_sb = {}
            for nm, bd in (("q", bq_d), ("k", bk_d), ("v", bv_d)):
                t = persist.tile([128, 2], F32, name=f"b{nm}_sb")
                nc.sync.dma_start(t[:], bd.rearrange("(hp p) -> p hp", p=128))
                b_sb[nm] = t
            nc.sync.dma_start(
                woT_sb[:], woT_d.rearrange("(hh p) j -> p hh j", p=64)
            )

            # ---------------- Stage A: transpose + projections ----------------
            with contextlib.ExitStack() as actx:
                nat_pool = actx.enter_context(tc.tile_pool(name="nat", bufs=2))
                xt_pool = actx.enter_context(tc.tile_pool(name="xt", bufs=3))
                wT_pool = actx.enter_context(tc.tile_pool(name="wT", bufs=1))
                vtmp_pool = actx.enter_context(tc.tile_pool(name="vtmp", bufs=1))
                tp_ps = actx.enter_context(
                    tc.tile_pool(name="tp_ps", bufs=2, space="PSUM")
                )
                pr_ps = actx.enter_context(
                    tc.tile_pool(name="pr_ps", bufs=2, space="PSUM")
                )

                vT_tmp = [
                    vtmp_pool.tile([128, S], F32R, name=f"vT{hp}", tag=f"vT{hp}")
                    for hp in range(2)
                ]

                for x_d, wT_d2, bias_nm, dests in (
                    (xq_d, wqT_d, "q", qT),
                    (xk_d, wkT_d, "k", kT),
                    (xv_d, wvT_d, "v", vT_tmp),
                ):
                    wT_sb = wT_pool.tile([128, 8, HPC * DH], F32R, tag="wT", name="wT_sb")
                    nc.sync.dma_start(
                        wT_sb[:], wT_d2.rearrange("(dt p) c -> p dt c", p=128)
                    )
                    for sc in range(4):  # 512-row s-chunks
                        nat = nat_pool.tile([128, 4, D], F32R, tag="nat", name="nat")
                        nc.sync.dma_start(
                            nat[:],
                            x_d[sc * 512 : (sc + 1) * 512, :].rearrange(
                                "(so p) d -> p so d", p=128
                            ),
                        )
                        pps = [
                            pr_ps.tile([128, 512], F32, tag="pr", name="pps")
                            for _ in range(2)
                        ]
                        for dt in range(8):
                            tp = tp_ps.tile([128, 512], F32R, tag="tp", name="tp")
                            for so in range(4):
                                nc.tensor.transpose(
                                    tp[:, so * 128 : (so + 1) * 128],
                                    nat[:, so, dt * 128 : (dt + 1) * 128],
                                    ident_r[:],
                                )
                            xt = xt_pool.tile([128, 512], F32R, tag="xt", name="xt")
                            nc.vector.tensor_copy(xt[:], tp[:])
                            for hp in range(2):
                                nc.tensor.matmul(
                                    pps[hp][:],
                                    wT_sb[:, dt, hp * 128 : (hp + 1) * 128],
                                    xt[:],
                                    start=(dt == 0),
                                    stop=(dt == 7),
                                )
                        for hp in range(2):
                            nc.vector.tensor_scalar_add(
                                dests[hp][:, sc * 512 : (sc + 1) * 512],
                                pps[hp][:],
                                b_sb[bias_nm][:, hp : hp + 1],
                            )

                # v: second transpose into natural [seq, head_dim] layout
                for hp in range(2):
                    for st in range(S // 128):
                        tpv = tp_ps.tile([128, 512], F32R, tag="tp", name="tpv")
                        nc.tensor.transpose(
                            tpv[:, :128],
                            vT_tmp[hp][:, st * 128 : (st + 1) * 128],
                            ident_r[:],
                        )
                        nc.vector.tensor_copy(
                            v_sb[:, st * 256 + hp * 128 : st * 256 + (hp + 1) * 128],
                            tpv[:, :128],
                        )

            # ---------------- Stage B: attention per head ----------------
            with contextlib.ExitStack() as bctx:
                p_pool = bctx.enter_context(tc.tile_pool(name="p", bufs=3))
                pT_pool = bctx.enter_context(tc.tile_pool(name="pT", bufs=3))
                sm_pool = bctx.enter_context(tc.tile_pool(name="sm", bufs=8))
                avsb_pool = bctx.enter_context(tc.tile_pool(name="avsb", bufs=2))
                avn_pool = bctx.enter_context(tc.tile_pool(name="avn", bufs=3))
                out_pool = bctx.enter_context(tc.tile_pool(name="outp", bufs=2))
                sc_ps = bctx.enter_context(
                    tc.tile_pool(name="sc_ps", bufs=2, space="PSUM")
                )
                av_ps_pool = bctx.enter_context(
                    tc.tile_pool(name="av_ps", bufs=1, space="PSUM")
                )

                for h in range(HPC):
                    hp, off = h // 2, (h % 2) * 64
                    qTh = qT[hp][off : off + 64, :]
                    kTh = kT[hp][off : off + 64, :]

                    # Phase 1: attn output rows
                    for qt in range(NQT):
                        p_sb = p_pool.tile([128, S], F32, tag="p", name="p_sb")
                        sums2 = sm_pool.tile([128, 2], F32, tag="sm", name="sums2")
                        for half in range(2):
                            sps = sc_ps.tile([128, 1024], F32, tag="sc", name="sps")
                            for nk in range(2):
                                nc.tensor.matmul(
                                    sps[:, nk * 512 : (nk + 1) * 512],
                                    qTh[:, qt * 128 : (qt + 1) * 128],
                                    kTh[:, half * 1024 + nk * 512 : half * 1024 + (nk + 1) * 512],
                                    start=True,
                                    stop=True,
                                )
                            nc.scalar.activation(
                                p_sb[:, half * 1024 : (half + 1) * 1024],
                                sps[:],
                                EXP,
                                accum_out=sums2[:, half : half + 1],
                            )
                        tot = sm_pool.tile([128, 1], F32, tag="tot", name="tot")
                        nc.vector.tensor_add(tot[:], sums2[:, 0:1], sums2[:, 1:2])
                        rs = rsum[:, h * NQT + qt : h * NQT + qt + 1]
                        nc.vector.reciprocal(rs, tot[:])
                        nc.vector.tensor_scalar_mul(p_sb[:], p_sb[:], rs)
                        nc.sync.dma_start(
                            attn_d[h, qt * 128 : (qt + 1) * 128, :], p_sb[:]
                        )

                    # Phase 2: PV product (unnormalized), av in [64, S] PSUM
                    av_ps = av_ps_pool.tile([64, S], F32, tag="av", name="av_ps")
                    for kt in range(S // 128):
                        pT = pT_pool.tile([128, S], F32R, tag="pT", name="pT")
                        for half in range(2):
                            sps = sc_ps.tile([128, 1024], F32, tag="sc", name="spsT")
                            for nq in range(2):
                                nc.tensor.matmul(
                                    sps[:, nq * 512 : (nq + 1) * 512],
                                    kTh[:, kt * 128 : (kt + 1) * 128],
                                    qTh[:, half * 1024 + nq * 512 : half * 1024 + (nq + 1) * 512],
                                    start=True,
                                    stop=True,
                                )
                            nc.scalar.activation(
                                pT[:, half * 1024 : (half + 1) * 1024], sps[:], EXP
                            )
                        vcol = kt * 256 + hp * 128 + (h % 2) * 64
                        for qc in range(4):
                            nc.tensor.matmul(
                                av_ps[:, qc * 512 : (qc + 1) * 512],
                                v_sb[:, vcol : vcol + 64],
                                pT[:, qc * 512 : (qc + 1) * 512],
                                start=(kt == 0),
                                stop=(kt == S // 128 - 1),
                                skip_group_check=True,
                            )

                    # normalize av rows by 1/rowsum via transpose round-trip
                    av_sb = avsb_pool.tile([64, S], F32R, tag="avsb", name="av_sb")
                    nc.vector.tensor_copy(av_sb[:], av_ps[:])
                    for qt in range(NQT):
                        t1 = sc_ps.tile([128, 1024], F32R, tag="sc", name="t1")
                        nc.tensor.transpose(
                            t1[:, :64],
                            av_sb[:, qt * 128 : (qt + 1) * 128],
                            ident_r[:64, :64],
                        )
                        avn = avn_pool.tile([128, 64], F32R, tag="avn", name="avn")
                        nc.vector.tensor_scalar_mul(
                            avn[:],
                            t1[:, :64],
                            rsum[:, h * NQT + qt : h * NQT + qt + 1],
                        )
                        t2 = sc_ps.tile([128, 1024], F32R, tag="sc", name="t2")
                        nc.tensor.transpose(
                            t2[:64, :128],
                            avn[:],
                            ident_r[:],
                        )
                        nc.vector.tensor_copy(
                            avT[h][:, qt * 128 : (qt + 1) * 128],
                            t2[:64, :128],
                        )

                # ---------------- Stage C: output projection ----------------
                for qt in range(NQT):
                    wps = sc_ps.tile([128, 1024], F32, tag="sc", name="wps")
                    for hh in range(HPC):
                        for nj in range(2):
                            nc.tensor.matmul(
                                wps[:, nj * 512 : (nj + 1) * 512],
                                avT[hh][:, qt * 128 : (qt + 1) * 128],
                                woT_sb[:, hh, nj * 512 : (nj + 1) * 512],
                                start=(hh == 0),
                                stop=(hh == HPC - 1),
                            )
                    out_sb = out_pool.tile([128, D], F32, tag="out", name="out_sb")
                    nc.vector.tensor_copy(out_sb[:], wps[:])
                    nc.sync.dma_start(pout_d[qt * 128 : (qt + 1) * 128, :], out_sb[:])

    nc.compile()
    return nc


def _get_nc():
    if "nc" not in _CACHE:
        _CACHE["nc"] = _build()
    return _CACHE["nc"]


def _in_maps(query, key, value, Wq, bq, Wk, bk, Wv, bv, Wo, bo):
    maps = []
    for c in range(N_CORES):
        b = c // 4
        h0 = (c % 4) * HPC
        cols = slice(h0 * DH, (h0 + HPC) * DH)
        maps.append(
            {
                "xq": np.ascontiguousarray(query[b]),
                "xk": np.ascontiguousarray(key[b]),
                "xv": np.ascontiguousarray(value[b]),
                "wqT": np.ascontiguousarray((Wq[cols] * SCALE).T),
                "wkT": np.ascontiguousarray(Wk[cols].T),
                "wvT": np.ascontiguousarray(Wv[cols].T),
                "woT": np.ascontiguousarray(Wo[:, cols].T),
                "bq": np.ascontiguousarray(bq[cols] * SCALE),
                "bk": np.ascontiguousarray(bk[cols]),
                "bv": np.ascontiguousarray(bv[cols]),
            }
        )
    return maps


def _get_runner(reps=1):
    """Cached jitted SPMD runner (replicates bass2jax multi-core path, but
    with a persistent jit so repeat calls don't retrace, and optional
    chained repetitions for timing)."""
    key = ("runner", reps)
    if key in _CACHE:
        return _CACHE[key]

    import jax
    import concourse.mybir as mybir
    from jax.experimental.shard_map import shard_map
    from jax.sharding import Mesh, NamedSharding, PartitionSpec
    from concourse.bass2jax import (
        _bass_exec_p,
        install_neuronx_cc_hook,
        partition_id_tensor,
    )

    install_neuronx_cc_hook()
    nc = _get_nc()

    partition_name = nc.partition_id_tensor.name if nc.partition_id_tensor else None
    in_names, out_names, out_avals, zero_outs = [], [], [], []
    for alloc in nc.m.functions[0].allocations:
        if not isinstance(alloc, mybir.MemoryLocationSet):
            continue
        name = alloc.memorylocations[0].name
        if alloc.kind == "ExternalInput":
            if name != partition_name:
                in_names.append(name)
        elif alloc.kind == "ExternalOutput":
            out_names.append(name)
            shape = tuple(alloc.tensor_shape)
            dtype = mybir.dt.np(alloc.dtype)
            out_avals.append(jax.core.ShapedArray(shape, dtype))
            zero_outs.append(np.zeros((N_CORES * shape[0], *shape[1:]), dtype))
    n_params = len(in_names)
    all_names = in_names + out_names
    if partition_name is not None:
        all_names = all_names + [partition_name]

    def _body(*args):
        ins = list(args[:n_params])
        outs = list(args[n_params:])
        for _ in range(reps):
            operands = ins + outs
            if partition_name is not None:
                operands.append(partition_id_tensor())
            outs = list(
                _bass_exec_p.bind(
                    *operands,
                    out_avals=tuple(out_avals),
                    in_names=tuple(all_names),
                    out_names=tuple(out_names),
                    lowering_input_output_aliases=(),
                    sim_require_finite=True,
                    sim_require_nnan=True,
                    nc=nc,
                )
            )
        return tuple(outs)

    devices = jax.devices()[:N_CORES]
    mesh = Mesh(np.asarray(devices), ("core",))
    nouts = len(out_names)
    sharded = jax.jit(
        shard_map(
            _body,
            mesh=mesh,
            in_specs=(PartitionSpec("core"),) * (n_params + nouts),
            out_specs=(PartitionSpec("core"),) * nouts,
            check_rep=False,
        ),
        keep_unused=True,
    )
    sh = NamedSharding(mesh, PartitionSpec("core"))
    zeros_dev = [jax.device_put(z, sh) for z in zero_outs]

    def prep(maps):
        concat_in = [
            np.concatenate([np.asarray(maps[c][nm]) for c in range(N_CORES)], axis=0)
            for nm in in_names
        ]
        return [jax.device_put(a, sh) for a in concat_in]

    def run(maps, as_numpy=True):
        concat_in = [
            np.concatenate([np.asarray(maps[c][nm]) for c in range(N_CORES)], axis=0)
            for nm in in_names
        ]
        out_arrs = sharded(*concat_in, *zeros_dev)
        if not as_numpy:
            return out_arrs
        return [
            {
                nm: np.asarray(out_arrs[i]).reshape(N_CORES, *out_avals[i].shape)[c]
                for i, nm in enumerate(out_names)
            }
            for c in range(N_CORES)
        ], out_names

    run.in_names = in_names
    run.out_names = out_names
    run.sharded = sharded
    run.zeros_dev = zeros_dev
    run.prep = prep
    _CACHE[key] = run
    return run


def kernel(query, key, value, Wq, bq, Wk, bk, Wv, bv, Wo, bo):
    args = [np.asarray(a, dtype=np.float32) for a in
            (query, key, value, Wq, bq, Wk, bk, Wv, bv, Wo, bo)]
    query, key, value, Wq, bq, Wk, bk, Wv, bv, Wo, bo = args

    maps = _in_maps(query, key, value, Wq, bq, Wk, bk, Wv, bv, Wo, bo)
    results, _ = _get_runner()(maps)

    attn = np.empty((B, H, S, S), dtype=np.float32)
    out = np.zeros((B, S, D), dtype=np.float32)
    for c in range(N_CORES):
        b = c // 4
        h0 = (c % 4) * HPC
        attn[b, h0 : h0 + HPC] = results[c]["attn"]
        out[b] += results[c]["pout"]
    out += bo
    return out, attn
